# revision 29
# baseline (speedup 1.0000x reference)
"""FM layer kernel for Trainium2, 8 NeuronCores — pair-packed dma_gather.

Data-parallel over batch (512 rows/core). Sparse embedding rows live in a
bf16 table at 512B/pair-unit: [A (65 bf16: V row 2u, c) | pad | D (65 bf16:
row 2u+1 - row 2u, c diff) | pad], c = w - 0.5||V||^2. Pair units keep a
field's 40000 rows inside the int16 dma_gather index window (20000 units).
One ext-isa dma_gather per (core, field) over the 4 SWDGE queues; fields
24/25 are split into 256-idx halves so every queue carries exactly 3328
descriptors, dispatched in queue order (the Pool engine issues in program
order and blocks while a queue-pair is busy). Q7 descriptor generation
(~9 ns/desc/queue-pair) is the phase bottleneck (~33 us).

Parity select on DVE: sel = A + M*D with M a [128,26,4,1] bf16 mask
broadcast along the 65-lane axis (c rides as lane 64, so one op chain
covers V and c). Groups of 4 fields pipeline behind the gathers into a
2-wide fp16 running accumulator; the last two fields are single-field
groups to shorten the post-last-gather critical chain. All inputs load on
the ACT HWDGE queue (gi first). Dense terms fold into [27,65] matmuls into
one PSUM tile; the final 0.5*sum sv^2 + c runs as DVE mul + segmented
reduce + scalar_tensor_tensor, then one [128,4] f32 output DMA.
"""
import numpy as np
import ml_dtypes

import concourse.bass as bass
import concourse.bacc as bacc
import concourse.mybir as mybir
import concourse.tile as tile
from concourse import bass_utils
from concourse.library_config import mlp

NUM_DENSE = 13
NUM_SPARSE = 26
FEAT = 40000
K = 64
ROW = K + 1             # 65: V lanes + c lane
BATCH = 4096
N_CORES = 8
BPC = BATCH // N_CORES  # 512
NT = 4
UNIT = 128              # bf16 elems per table row slot (256 B)
PAIRS = FEAT // 2       # 20000 pair units per field
TRP = NUM_SPARSE * PAIRS
NIDX = BPC
IFREE = NIDX // 16
CDIM = 2 * NUM_DENSE + 1
F32 = mybir.dt.float32
GRP4 = [(0, 4), (4, 8), (8, 12), (12, 16), (16, 20)]

TRACE = False
LAST = {}

_nc_cache = []
_tab_cache = []


def _build():
    f32 = mybir.dt.float32
    bf16 = mybir.dt.bfloat16
    i16 = mybir.dt.int16
    nc = bacc.Bacc("TRN2", target_bir_lowering=False, debug=False,
                   num_devices=N_CORES, num_swdge_queues=4)
    tab_d = nc.dram_tensor("tab", [TRP, 2 * UNIT], bf16, kind="ExternalInput").ap()
    gi_d = nc.dram_tensor("gi", [128, NUM_SPARSE * IFREE], i16, kind="ExternalInput").ap()
    m_d = nc.dram_tensor("m", [128, NUM_SPARSE * NT], bf16, kind="ExternalInput").ap()
    dn_d = nc.dram_tensor("dn", [CDIM, BPC + ROW], f32, kind="ExternalInput").ap()
    y_d = nc.dram_tensor("y", [128, NT], f32, kind="ExternalOutput").ap()

    with tile.TileContext(nc) as tc:
        with (
            tc.tile_pool(name="xp", bufs=1) as xp,
            tc.tile_pool(name="sp", bufs=1) as sp,
            tc.tile_pool(name="pp", bufs=2, space="PSUM") as pp,
        ):
            nc.gpsimd.load_library(mlp)

            gi = sp.tile([128, NUM_SPARSE, IFREE], i16, tag="gi")
            nc.scalar.dma_start(gi[:], gi_d[:, :])
            mm = sp.tile([128, NUM_SPARSE, NT, 1], bf16, tag="mm")
            nc.scalar.dma_start(mm[:], m_d[:, :])
            dn_sb = sp.tile([CDIM, BPC + ROW], f32, tag="dn")
            nc.scalar.dma_start(dn_sb[:], dn_d[:, :])
            lhs_sb = dn_sb[:, 0:BPC]
            rhs_sb = dn_sb[:, BPC:BPC + ROW]

            r512 = nc.gpsimd.to_reg(NIDX)
            r256 = nc.gpsimd.to_reg(NIDX // 2)
            x = xp.tile([128, NUM_SPARSE, NT, 2 * UNIT], bf16, tag="x")

            def half_gather(s, half, q):
                nc.gpsimd.dma_gather(
                    x[:, s, 2 * half:2 * half + 2],
                    tab_d[s * PAIRS:(s + 1) * PAIRS, :],
                    gi[:, s, half * (IFREE // 2):(half + 1) * (IFREE // 2)],
                    NIDX // 2, r256, 2 * UNIT, queue_num=q,
                )

            # The Pool engine dispatches in program order and the gang blocks
            # for the full descgen of the oldest in-flight instruction, so
            # queues 1-3 idle until queue 0's first gather retires. Lead with
            # 256-idx half-gathers (2.3us blockers instead of 4.6us), keep
            # fields 0..21 round-robin, and land fields 22/23 as half-gathers
            # at the end so the tail group is small. Every queue carries
            # exactly 3328 descriptors.
            half_gather(24, 0, 0)
            half_gather(24, 1, 1)
            half_gather(25, 0, 2)
            half_gather(25, 1, 3)
            for s in range(22):
                nc.gpsimd.dma_gather(
                    x[:, s], tab_d[s * PAIRS:(s + 1) * PAIRS, :], gi[:, s],
                    NIDX, r512, 2 * UNIT, queue_num=s % 4,
                )
            half_gather(22, 0, 2)
            half_gather(22, 1, 3)
            half_gather(23, 0, 2)
            half_gather(23, 1, 3)

            # dense-term matmuls run early (only need dn); one PSUM tile
            psum = pp.tile([128, NT, ROW], f32, tag="ps", space="PSUM")
            for t in range(NT):
                nc.tensor.matmul(
                    out=psum[:, t], lhsT=lhs_sb[:, t * 128:(t + 1) * 128],
                    rhs=rhs_sb[:], start=True, stop=True,
                )

            # sel = A + M*D; A = x[...,0:65], D = x[...,128:193] (bf16),
            # M broadcast along the lane axis. 7 groups pipeline behind the
            # gathers; a 2-wide running accumulator replaces the final tree.
            # PAD = 128-lane pitch keeps every DVE row 256B-aligned.
            PAD = 2 * ROW - 2  # 128
            f16 = mybir.dt.float16
            md = xp.tile([128, NUM_SPARSE, NT, PAD], f16, tag="md")
            l1 = xp.tile([128, NUM_SPARSE, NT, PAD], f16, tag="l1")
            t2 = xp.tile([128, 5, 2, NT, PAD], f16, tag="t2")
            racc = xp.tile([128, 6, 2, NT, PAD], f16, tag="racc")
            s32a = xp.tile([128, NT, PAD], f16, tag="s32a")
            s32b = xp.tile([128, NT, PAD], f16, tag="s32b")
            s32 = sp.tile([128, NT, ROW], f32, tag="s32")

            def sel(lo, hi):
                n = hi - lo
                sl = slice(lo, hi)
                mb = mm[:, sl].broadcast_to([128, n, NT, ROW])
                nc.vector.tensor_mul(
                    md[:, sl, :, 0:ROW], mb, x[:, sl, :, UNIT:UNIT + ROW]
                )
                nc.vector.tensor_add(
                    l1[:, sl, :, 0:ROW], x[:, sl, :, 0:ROW], md[:, sl, :, 0:ROW]
                )

            # arrival order: 24/25 (lead halves), 0..19 (rounds), 20/21, 22, 23
            sel(24, 26)
            for g, (lo, hi) in enumerate(GRP4):
                sel(lo, hi)
                nc.vector.tensor_add(
                    t2[:, g, :, :, 0:ROW],
                    l1[:, lo:lo + 2, :, 0:ROW], l1[:, lo + 2:lo + 4, :, 0:ROW],
                )
                nc.vector.tensor_add(
                    racc[:, g, :, :, 0:ROW],
                    l1[:, 24:26, :, 0:ROW] if g == 0 else racc[:, g - 1, :, :, 0:ROW],
                    t2[:, g, :, :, 0:ROW],
                )
            sel(20, 22)
            nc.vector.tensor_add(
                racc[:, 5, :, :, 0:ROW],
                racc[:, 4, :, :, 0:ROW], l1[:, 20:22, :, 0:ROW],
            )
            nc.vector.tensor_add(
                s32a[:, :, 0:ROW],
                racc[:, 5, 0, :, 0:ROW], racc[:, 5, 1, :, 0:ROW],
            )
            sel(22, 23)
            nc.vector.tensor_add(
                s32b[:, :, 0:ROW], s32a[:, :, 0:ROW], l1[:, 22, :, 0:ROW]
            )
            sel(23, 24)
            nc.vector.tensor_add(
                s32[:], s32b[:, :, 0:ROW], l1[:, 23, :, 0:ROW]
            )

            # out = 0.5*sum_k tot_k^2 + tot_c, all on DVE (4 ops total)
            tot = sp.tile([128, NT, ROW], f32, tag="tot")
            sq = sp.tile([128, NT, K], f32, tag="sq")
            red = sp.tile([128, NT, 1], f32, tag="red")
            o = sp.tile([128, NT, 1], f32, tag="o")
            nc.vector.tensor_add(tot[:], s32[:], psum[:])
            nc.vector.tensor_mul(sq[:], tot[:, :, 0:K], tot[:, :, 0:K])
            nc.vector.tensor_reduce(
                red[:], sq[:], mybir.AxisListType.X, mybir.AluOpType.add
            )
            nc.vector.scalar_tensor_tensor(
                out=o[:], in0=red[:], scalar=0.5, in1=tot[:, :, K:K + 1],
                op0=mybir.AluOpType.mult, op1=mybir.AluOpType.add,
            )
            nc.sync.dma_start(y_d[:, :], o[:, :, 0])
    nc.compile()
    return nc


def _build_table(w, V):
    """[TRP, 256] bf16 pair units: [A 65 | pad | D 65 | pad]."""
    Vt = np.ascontiguousarray(V.T)  # [F, K] f32
    c = (w[:, 0] - 0.5 * np.einsum("fk,fk->f", Vt, Vt)).astype(np.float32)
    nrows = NUM_SPARSE * FEAT
    va = np.empty((nrows, ROW), dtype=np.float32)
    va[:, 0:K] = Vt[NUM_DENSE:NUM_DENSE + nrows]
    va[:, K] = c[NUM_DENSE:NUM_DENSE + nrows]
    va = va.reshape(TRP, 2, ROW)
    tab = np.zeros((TRP, 2 * UNIT), dtype=ml_dtypes.bfloat16)
    tab[:, 0:ROW] = va[:, 0].astype(ml_dtypes.bfloat16)
    tab[:, UNIT:UNIT + ROW] = (va[:, 1] - va[:, 0]).astype(ml_dtypes.bfloat16)
    return np.ascontiguousarray(tab)


def _prepare_dense(dense, w0, w, V):
    Vt_d = np.ascontiguousarray(V[:, :NUM_DENSE].T)  # [13, K]
    lhs = np.concatenate(
        [dense.T, dense.T ** 2, np.ones((1, BATCH), np.float32)], axis=0
    ).astype(np.float32)
    rhs = np.zeros((CDIM, ROW), dtype=np.float32)
    rhs[0:NUM_DENSE, 0:K] = Vt_d
    rhs[0:NUM_DENSE, K] = w[:NUM_DENSE, 0]
    rhs[NUM_DENSE:2 * NUM_DENSE, K] = -0.5 * (Vt_d ** 2).sum(axis=1)
    rhs[2 * NUM_DENSE, K] = np.asarray(w0).reshape(-1)[0]
    return lhs, rhs


def _prepare_idx(sparse):
    i = sparse.astype(np.int32)            # [BATCH, 26]
    pair = (i >> 1).astype(np.int16)
    par = (i & 1)
    gi_all, m_all = [], []
    for cidx in range(N_CORES):
        sl = slice(cidx * BPC, (cidx + 1) * BPC)
        cols = [
            np.tile(pair[sl, s].reshape(IFREE, 16).T, (8, 1))
            for s in range(NUM_SPARSE)
        ]
        gi_all.append(np.ascontiguousarray(np.concatenate(cols, axis=1)))
        # mask [128, 26, 4]: slot (p, s, t) = parity of batch row t*128+p
        pc = par[sl].reshape(NT, 128, NUM_SPARSE).transpose(1, 2, 0)
        m_all.append(np.ascontiguousarray(
            pc.astype(ml_dtypes.bfloat16).reshape(128, -1)))
    return gi_all, m_all


def kernel(dense_inputs, sparse_inputs, w0, w, V):
    dense = np.asarray(dense_inputs, dtype=np.float32)
    sparse = np.asarray(sparse_inputs)
    w0 = np.asarray(w0, dtype=np.float32)
    w = np.asarray(w, dtype=np.float32)
    V = np.asarray(V, dtype=np.float32)

    if not _nc_cache:
        _nc_cache.append(_build())
    nc = _nc_cache[0]
    fp = (w[:3, 0].tobytes(), V[:2, :3].tobytes(), float(w.sum()))
    if not _tab_cache or _tab_cache[0][0] != fp:
        _tab_cache[:] = [(fp, _build_table(w, V))]
    tab = _tab_cache[0][1]

    lhs, rhs = _prepare_dense(dense, w0, w, V)
    gi_all, m_all = _prepare_idx(sparse)

    in_maps = []
    for c in range(N_CORES):
        sl = slice(c * BPC, (c + 1) * BPC)
        dn = np.concatenate([lhs[:, sl], rhs], axis=1)
        in_maps.append({
            "tab": tab,
            "gi": gi_all[c],
            "m": m_all[c],
            "dn": np.ascontiguousarray(dn),
        })
    res = bass_utils.run_bass_kernel_spmd(
        nc, in_maps, core_ids=list(range(N_CORES)), trace=TRACE
    )
    LAST["res"] = res
    # y[p, t] on core c -> out[c*512 + t*128 + p]
    out = np.concatenate(
        [res.results[c]["y"].T.reshape(BPC, 1) for c in range(N_CORES)], axis=0
    )
    return out.astype(np.float32)


# revision 30
# speedup vs baseline: 1.0064x; 1.0064x over previous
"""FM layer kernel for Trainium2, 8 NeuronCores — pair-packed dma_gather.

Data-parallel over batch (512 rows/core). Sparse embedding rows live in a
bf16 table at 512B/pair-unit: [A (65 bf16: V row 2u, c) | pad | D (65 bf16:
row 2u+1 - row 2u, c diff) | pad], c = w - 0.5||V||^2. Pair units keep a
field's 40000 rows inside the int16 dma_gather index window (20000 units).
One ext-isa dma_gather per (core, field) over the 4 SWDGE queues; fields
24/25 are split into 256-idx halves so every queue carries exactly 3328
descriptors, dispatched in queue order (the Pool engine issues in program
order and blocks while a queue-pair is busy). Q7 descriptor generation
(~9 ns/desc/queue-pair) is the phase bottleneck (~33 us).

Parity select on DVE: sel = A + M*D with M a [128,26,4,1] bf16 mask
broadcast along the 65-lane axis (c rides as lane 64, so one op chain
covers V and c). Groups of 4 fields pipeline behind the gathers into a
2-wide fp16 running accumulator; the last two fields are single-field
groups to shorten the post-last-gather critical chain. All inputs load on
the ACT HWDGE queue (gi first). Dense terms fold into [27,65] matmuls into
one PSUM tile; the final 0.5*sum sv^2 + c runs as DVE mul + segmented
reduce + scalar_tensor_tensor, then one [128,4] f32 output DMA.
"""
import numpy as np
import ml_dtypes

import concourse.bass as bass
import concourse.bacc as bacc
import concourse.mybir as mybir
import concourse.tile as tile
from concourse import bass_utils
from concourse.library_config import mlp

NUM_DENSE = 13
NUM_SPARSE = 26
FEAT = 40000
K = 64
ROW = K + 1             # 65: V lanes + c lane
BATCH = 4096
N_CORES = 8
BPC = BATCH // N_CORES  # 512
NT = 4
UNIT = 128              # bf16 elems per table row slot (256 B)
PAIRS = FEAT // 2       # 20000 pair units per field
TRP = NUM_SPARSE * PAIRS
NIDX = BPC
IFREE = NIDX // 16
CDIM = 2 * NUM_DENSE + 1
F32 = mybir.dt.float32
GRP4 = [(0, 4), (4, 8), (8, 12), (12, 16), (16, 20)]

TRACE = False
LAST = {}

_nc_cache = []
_tab_cache = []


def _build():
    f32 = mybir.dt.float32
    bf16 = mybir.dt.bfloat16
    i16 = mybir.dt.int16
    nc = bacc.Bacc("TRN2", target_bir_lowering=False, debug=False,
                   num_devices=N_CORES, num_swdge_queues=4)
    tab_d = nc.dram_tensor("tab", [TRP, 2 * UNIT], bf16, kind="ExternalInput").ap()
    gi_d = nc.dram_tensor("gi", [128, NUM_SPARSE * IFREE], i16, kind="ExternalInput").ap()
    m_d = nc.dram_tensor("m", [128, NUM_SPARSE * NT], bf16, kind="ExternalInput").ap()
    dn_d = nc.dram_tensor("dn", [CDIM, BPC + ROW], f32, kind="ExternalInput").ap()
    y_d = nc.dram_tensor("y", [128, NT], f32, kind="ExternalOutput").ap()

    with tile.TileContext(nc) as tc:
        with (
            tc.tile_pool(name="xp", bufs=1) as xp,
            tc.tile_pool(name="sp", bufs=1) as sp,
            tc.tile_pool(name="pp", bufs=2, space="PSUM") as pp,
        ):
            nc.gpsimd.load_library(mlp)

            gi = sp.tile([128, NUM_SPARSE, IFREE], i16, tag="gi")
            nc.scalar.dma_start(gi[:], gi_d[:, :])
            mm = sp.tile([128, NUM_SPARSE, NT, 1], bf16, tag="mm")
            nc.scalar.dma_start(mm[:], m_d[:, :])
            dn_sb = sp.tile([CDIM, BPC + ROW], f32, tag="dn")
            nc.scalar.dma_start(dn_sb[:], dn_d[:, :])
            lhs_sb = dn_sb[:, 0:BPC]
            rhs_sb = dn_sb[:, BPC:BPC + ROW]

            r512 = nc.gpsimd.to_reg(NIDX)
            r256 = nc.gpsimd.to_reg(NIDX // 2)
            x = xp.tile([128, NUM_SPARSE, NT, 2 * UNIT], bf16, tag="x")

            def half_gather(s, half, q):
                nc.gpsimd.dma_gather(
                    x[:, s, 2 * half:2 * half + 2],
                    tab_d[s * PAIRS:(s + 1) * PAIRS, :],
                    gi[:, s, half * (IFREE // 2):(half + 1) * (IFREE // 2)],
                    NIDX // 2, r256, 2 * UNIT, queue_num=q,
                )

            # The Pool engine dispatches in program order and blocks on the
            # oldest in-flight instruction, so queues 1-3 idle until queue 0's
            # first gather retires. Lead with 256-idx half-gathers of fields
            # 24/25 (2.3us blocker instead of 4.6us, one half per queue), then
            # fields 0..23 round-robin. Every queue carries 3328 descriptors.
            half_gather(24, 0, 0)
            half_gather(25, 0, 1)
            half_gather(24, 1, 2)
            half_gather(25, 1, 3)
            for s in range(24):
                nc.gpsimd.dma_gather(
                    x[:, s], tab_d[s * PAIRS:(s + 1) * PAIRS, :], gi[:, s],
                    NIDX, r512, 2 * UNIT, queue_num=s % 4,
                )

            # dense-term matmuls run early (only need dn); one PSUM tile
            psum = pp.tile([128, NT, ROW], f32, tag="ps", space="PSUM")
            for t in range(NT):
                nc.tensor.matmul(
                    out=psum[:, t], lhsT=lhs_sb[:, t * 128:(t + 1) * 128],
                    rhs=rhs_sb[:], start=True, stop=True,
                )

            # sel = A + M*D; A = x[...,0:65], D = x[...,128:193] (bf16),
            # M broadcast along the lane axis. 7 groups pipeline behind the
            # gathers; a 2-wide running accumulator replaces the final tree.
            # PAD = 128-lane pitch keeps every DVE row 256B-aligned.
            PAD = 2 * ROW - 2  # 128
            f16 = mybir.dt.float16
            md = xp.tile([128, NUM_SPARSE, NT, PAD], f16, tag="md")
            l1 = xp.tile([128, NUM_SPARSE, NT, PAD], f16, tag="l1")
            t2 = xp.tile([128, 5, 2, NT, PAD], f16, tag="t2")
            racc = xp.tile([128, 6, 2, NT, PAD], f16, tag="racc")
            s32a = xp.tile([128, NT, PAD], f16, tag="s32a")
            s32b = xp.tile([128, NT, PAD], f16, tag="s32b")
            s32 = sp.tile([128, NT, ROW], f32, tag="s32")

            def sel(lo, hi):
                n = hi - lo
                sl = slice(lo, hi)
                mb = mm[:, sl].broadcast_to([128, n, NT, ROW])
                nc.vector.tensor_mul(
                    md[:, sl, :, 0:ROW], mb, x[:, sl, :, UNIT:UNIT + ROW]
                )
                nc.vector.tensor_add(
                    l1[:, sl, :, 0:ROW], x[:, sl, :, 0:ROW], md[:, sl, :, 0:ROW]
                )

            # arrival order: 24/25 (lead halves), 0..19 (rounds), 20/21, 22, 23
            sel(24, 26)
            for g, (lo, hi) in enumerate(GRP4):
                sel(lo, hi)
                nc.vector.tensor_add(
                    t2[:, g, :, :, 0:ROW],
                    l1[:, lo:lo + 2, :, 0:ROW], l1[:, lo + 2:lo + 4, :, 0:ROW],
                )
                nc.vector.tensor_add(
                    racc[:, g, :, :, 0:ROW],
                    l1[:, 24:26, :, 0:ROW] if g == 0 else racc[:, g - 1, :, :, 0:ROW],
                    t2[:, g, :, :, 0:ROW],
                )
            sel(20, 22)
            nc.vector.tensor_add(
                racc[:, 5, :, :, 0:ROW],
                racc[:, 4, :, :, 0:ROW], l1[:, 20:22, :, 0:ROW],
            )
            nc.vector.tensor_add(
                s32a[:, :, 0:ROW],
                racc[:, 5, 0, :, 0:ROW], racc[:, 5, 1, :, 0:ROW],
            )
            sel(22, 23)
            nc.vector.tensor_add(
                s32b[:, :, 0:ROW], s32a[:, :, 0:ROW], l1[:, 22, :, 0:ROW]
            )
            sel(23, 24)
            nc.vector.tensor_add(
                s32[:], s32b[:, :, 0:ROW], l1[:, 23, :, 0:ROW]
            )

            # out = 0.5*sum_k tot_k^2 + tot_c, all on DVE (4 ops total)
            tot = sp.tile([128, NT, ROW], f32, tag="tot")
            sq = sp.tile([128, NT, K], f32, tag="sq")
            red = sp.tile([128, NT, 1], f32, tag="red")
            o = sp.tile([128, NT, 1], f32, tag="o")
            nc.vector.tensor_add(tot[:], s32[:], psum[:])
            nc.vector.tensor_mul(sq[:], tot[:, :, 0:K], tot[:, :, 0:K])
            nc.vector.tensor_reduce(
                red[:], sq[:], mybir.AxisListType.X, mybir.AluOpType.add
            )
            nc.vector.scalar_tensor_tensor(
                out=o[:], in0=red[:], scalar=0.5, in1=tot[:, :, K:K + 1],
                op0=mybir.AluOpType.mult, op1=mybir.AluOpType.add,
            )
            nc.sync.dma_start(y_d[:, :], o[:, :, 0])
    nc.compile()
    return nc


def _build_table(w, V):
    """[TRP, 256] bf16 pair units: [A 65 | pad | D 65 | pad]."""
    Vt = np.ascontiguousarray(V.T)  # [F, K] f32
    c = (w[:, 0] - 0.5 * np.einsum("fk,fk->f", Vt, Vt)).astype(np.float32)
    nrows = NUM_SPARSE * FEAT
    va = np.empty((nrows, ROW), dtype=np.float32)
    va[:, 0:K] = Vt[NUM_DENSE:NUM_DENSE + nrows]
    va[:, K] = c[NUM_DENSE:NUM_DENSE + nrows]
    va = va.reshape(TRP, 2, ROW)
    tab = np.zeros((TRP, 2 * UNIT), dtype=ml_dtypes.bfloat16)
    tab[:, 0:ROW] = va[:, 0].astype(ml_dtypes.bfloat16)
    tab[:, UNIT:UNIT + ROW] = (va[:, 1] - va[:, 0]).astype(ml_dtypes.bfloat16)
    return np.ascontiguousarray(tab)


def _prepare_dense(dense, w0, w, V):
    Vt_d = np.ascontiguousarray(V[:, :NUM_DENSE].T)  # [13, K]
    lhs = np.concatenate(
        [dense.T, dense.T ** 2, np.ones((1, BATCH), np.float32)], axis=0
    ).astype(np.float32)
    rhs = np.zeros((CDIM, ROW), dtype=np.float32)
    rhs[0:NUM_DENSE, 0:K] = Vt_d
    rhs[0:NUM_DENSE, K] = w[:NUM_DENSE, 0]
    rhs[NUM_DENSE:2 * NUM_DENSE, K] = -0.5 * (Vt_d ** 2).sum(axis=1)
    rhs[2 * NUM_DENSE, K] = np.asarray(w0).reshape(-1)[0]
    return lhs, rhs


def _prepare_idx(sparse):
    i = sparse.astype(np.int32)            # [BATCH, 26]
    pair = (i >> 1).astype(np.int16)
    par = (i & 1)
    gi_all, m_all = [], []
    for cidx in range(N_CORES):
        sl = slice(cidx * BPC, (cidx + 1) * BPC)
        cols = [
            np.tile(pair[sl, s].reshape(IFREE, 16).T, (8, 1))
            for s in range(NUM_SPARSE)
        ]
        gi_all.append(np.ascontiguousarray(np.concatenate(cols, axis=1)))
        # mask [128, 26, 4]: slot (p, s, t) = parity of batch row t*128+p
        pc = par[sl].reshape(NT, 128, NUM_SPARSE).transpose(1, 2, 0)
        m_all.append(np.ascontiguousarray(
            pc.astype(ml_dtypes.bfloat16).reshape(128, -1)))
    return gi_all, m_all


def kernel(dense_inputs, sparse_inputs, w0, w, V):
    dense = np.asarray(dense_inputs, dtype=np.float32)
    sparse = np.asarray(sparse_inputs)
    w0 = np.asarray(w0, dtype=np.float32)
    w = np.asarray(w, dtype=np.float32)
    V = np.asarray(V, dtype=np.float32)

    if not _nc_cache:
        _nc_cache.append(_build())
    nc = _nc_cache[0]
    fp = (w[:3, 0].tobytes(), V[:2, :3].tobytes(), float(w.sum()))
    if not _tab_cache or _tab_cache[0][0] != fp:
        _tab_cache[:] = [(fp, _build_table(w, V))]
    tab = _tab_cache[0][1]

    lhs, rhs = _prepare_dense(dense, w0, w, V)
    gi_all, m_all = _prepare_idx(sparse)

    in_maps = []
    for c in range(N_CORES):
        sl = slice(c * BPC, (c + 1) * BPC)
        dn = np.concatenate([lhs[:, sl], rhs], axis=1)
        in_maps.append({
            "tab": tab,
            "gi": gi_all[c],
            "m": m_all[c],
            "dn": np.ascontiguousarray(dn),
        })
    res = bass_utils.run_bass_kernel_spmd(
        nc, in_maps, core_ids=list(range(N_CORES)), trace=TRACE
    )
    LAST["res"] = res
    # y[p, t] on core c -> out[c*512 + t*128 + p]
    out = np.concatenate(
        [res.results[c]["y"].T.reshape(BPC, 1) for c in range(N_CORES)], axis=0
    )
    return out.astype(np.float32)


# revision 31
# speedup vs baseline: 1.0309x; 1.0243x over previous
"""FM layer kernel for Trainium2, 8 NeuronCores — pair-packed dma_gather.

Data-parallel over batch (512 rows/core). Sparse embedding rows live in a
bf16 table at 512B/pair-unit: [A (65 bf16: V row 2u, c) | pad | D (65 bf16:
row 2u+1 - row 2u, c diff) | pad], c = w - 0.5||V||^2. Pair units keep a
field's 40000 rows inside the int16 dma_gather index window (20000 units).
One ext-isa dma_gather per (core, field) over the 4 SWDGE queues; fields
24/25 are split into 256-idx halves so every queue carries exactly 3328
descriptors, dispatched in queue order (the Pool engine issues in program
order and blocks while a queue-pair is busy). Q7 descriptor generation
(~9 ns/desc/queue-pair) is the phase bottleneck (~33 us).

Parity select on DVE: sel = A + M*D with M a [128,26,4,1] bf16 mask
broadcast along the 65-lane axis (c rides as lane 64, so one op chain
covers V and c). Groups of 4 fields pipeline behind the gathers into a
2-wide fp16 running accumulator; the last two fields are single-field
groups to shorten the post-last-gather critical chain. All inputs load on
the ACT HWDGE queue (gi first). Dense terms fold into [27,65] matmuls into
one PSUM tile; the final 0.5*sum sv^2 + c runs as DVE mul + segmented
reduce + scalar_tensor_tensor, then one [128,4] f32 output DMA.
"""
import numpy as np
import ml_dtypes

import concourse.bass as bass
import concourse.bacc as bacc
import concourse.mybir as mybir
import concourse.tile as tile
from concourse import bass_utils
from concourse.library_config import mlp

NUM_DENSE = 13
NUM_SPARSE = 26
FEAT = 40000
K = 64
ROW = K + 1             # 65: V lanes + c lane
BATCH = 4096
N_CORES = 8
BPC = BATCH // N_CORES  # 512
NT = 4
UNIT = 128              # bf16 elems per table row slot (256 B)
PAIRS = FEAT // 2       # 20000 pair units per field
TRP = NUM_SPARSE * PAIRS
NIDX = BPC
IFREE = NIDX // 16
CDIM = 2 * NUM_DENSE + 1
F32 = mybir.dt.float32
GRP = [(0, 4), (4, 8), (8, 12), (12, 16), (16, 20), (20, 24), (24, 25), (25, 26)]

TRACE = False
LAST = {}

_nc_cache = []
_tab_cache = []


def _build():
    f32 = mybir.dt.float32
    bf16 = mybir.dt.bfloat16
    i16 = mybir.dt.int16
    nc = bacc.Bacc("TRN2", target_bir_lowering=False, debug=False,
                   num_devices=N_CORES, num_swdge_queues=4)
    tab_d = nc.dram_tensor("tab", [TRP, 2 * UNIT], bf16, kind="ExternalInput").ap()
    gi_d = nc.dram_tensor("gi", [128, NUM_SPARSE * IFREE], i16, kind="ExternalInput").ap()
    m_d = nc.dram_tensor("m", [128, NUM_SPARSE * NT], bf16, kind="ExternalInput").ap()
    dn_d = nc.dram_tensor("dn", [CDIM, BPC + ROW], f32, kind="ExternalInput").ap()
    y_d = nc.dram_tensor("y", [128, NT], f32, kind="ExternalOutput").ap()

    with tile.TileContext(nc) as tc:
        with (
            tc.tile_pool(name="xp", bufs=1) as xp,
            tc.tile_pool(name="sp", bufs=1) as sp,
            tc.tile_pool(name="pp", bufs=2, space="PSUM") as pp,
        ):
            nc.gpsimd.load_library(mlp)

            gi = sp.tile([128, NUM_SPARSE, IFREE], i16, tag="gi")
            nc.scalar.dma_start(gi[:], gi_d[:, :])
            mm = sp.tile([128, NUM_SPARSE, NT, 1], bf16, tag="mm")
            nc.scalar.dma_start(mm[:], m_d[:, :])
            dn_sb = sp.tile([CDIM, BPC + ROW], f32, tag="dn")
            nc.scalar.dma_start(dn_sb[:], dn_d[:, :])
            lhs_sb = dn_sb[:, 0:BPC]
            rhs_sb = dn_sb[:, BPC:BPC + ROW]

            r512 = nc.gpsimd.to_reg(NIDX)
            r256 = nc.gpsimd.to_reg(NIDX // 2)
            x = xp.tile([128, NUM_SPARSE, NT, 2 * UNIT], bf16, tag="x")
            for s in range(24):
                nc.gpsimd.dma_gather(
                    x[:, s], tab_d[s * PAIRS:(s + 1) * PAIRS, :], gi[:, s],
                    NIDX, r512, 2 * UNIT, queue_num=s % 4,
                )
            # fields 24/25 split by t-half so every queue carries 3328 descs.
            # Dispatch strictly in queue order q0,q1,q2,q3: the Pool engine
            # issues in program order and blocks while an instruction's
            # queue-pair is busy, so any other order head-of-line stalls.
            for s, half, q in ((24, 0, 0), (25, 0, 1), (24, 1, 2), (25, 1, 3)):
                nc.gpsimd.dma_gather(
                    x[:, s, 2 * half:2 * half + 2],
                    tab_d[s * PAIRS:(s + 1) * PAIRS, :],
                    gi[:, s, half * (IFREE // 2):(half + 1) * (IFREE // 2)],
                    NIDX // 2, r256, 2 * UNIT, queue_num=q,
                )

            # dense-term matmuls run early (only need dn); one PSUM tile
            psum = pp.tile([128, NT, ROW], f32, tag="ps", space="PSUM")
            for t in range(NT):
                nc.tensor.matmul(
                    out=psum[:, t], lhsT=lhs_sb[:, t * 128:(t + 1) * 128],
                    rhs=rhs_sb[:], start=True, stop=True,
                )

            # sel = A + M*D; A = x[...,0:65], D = x[...,128:193] (bf16),
            # M broadcast along the lane axis. 7 groups pipeline behind the
            # gathers; a 2-wide running accumulator replaces the final tree.
            # PAD = 128-lane pitch keeps every DVE row 256B-aligned.
            PAD = 2 * ROW - 2  # 128
            f16 = mybir.dt.float16
            md = xp.tile([128, NUM_SPARSE, NT, PAD], f16, tag="md")
            l1 = xp.tile([128, NUM_SPARSE, NT, PAD], f16, tag="l1")
            t2 = xp.tile([128, 7, 2, NT, PAD], f16, tag="t2")
            racc = xp.tile([128, 5, 2, NT, PAD], f16, tag="racc")
            s32a = xp.tile([128, NT, PAD], f16, tag="s32a")
            s32b = xp.tile([128, NT, PAD], f16, tag="s32b")
            s32 = sp.tile([128, NT, ROW], f32, tag="s32")
            for g, (lo, hi) in enumerate(GRP):
                sl = slice(lo, hi)
                n = hi - lo
                a = x[:, sl, :, 0:ROW]
                d = x[:, sl, :, UNIT:UNIT + ROW]
                mb = mm[:, sl].broadcast_to([128, n, NT, ROW])
                nc.vector.tensor_mul(md[:, sl, :, 0:ROW], mb, d)
                nc.vector.tensor_add(l1[:, sl, :, 0:ROW], a, md[:, sl, :, 0:ROW])
                if n == 4:
                    nc.vector.tensor_add(
                        t2[:, g, :, :, 0:ROW],
                        l1[:, lo:lo + 2, :, 0:ROW], l1[:, lo + 2:lo + 4, :, 0:ROW],
                    )
                # running accumulation of group pair-sums, pipelined with gathers
                if g == 1:
                    nc.vector.tensor_add(
                        racc[:, 0, :, :, 0:ROW],
                        t2[:, 0, :, :, 0:ROW], t2[:, 1, :, :, 0:ROW],
                    )
                elif 2 <= g <= 5:
                    nc.vector.tensor_add(
                        racc[:, g - 1, :, :, 0:ROW],
                        racc[:, g - 2, :, :, 0:ROW], t2[:, g, :, :, 0:ROW],
                    )
                    if g == 5:  # collapse the pair lanes while gathers finish
                        nc.vector.tensor_add(
                            s32a[:, :, 0:ROW],
                            racc[:, 4, 0, :, 0:ROW], racc[:, 4, 1, :, 0:ROW],
                        )
                elif g == 6:  # field 24 (single)
                    nc.vector.tensor_add(
                        s32b[:, :, 0:ROW], s32a[:, :, 0:ROW], l1[:, lo, :, 0:ROW]
                    )
                elif g == 7:  # field 25 (single) -> f32 total
                    nc.vector.tensor_add(
                        s32[:], s32b[:, :, 0:ROW], l1[:, lo, :, 0:ROW]
                    )

            # out = 0.5*sum_k tot_k^2 + tot_c, all on DVE (4 ops total)
            tot = sp.tile([128, NT, ROW], f32, tag="tot")
            sq = sp.tile([128, NT, K], f32, tag="sq")
            red = sp.tile([128, NT, 1], f32, tag="red")
            o = sp.tile([128, NT, 1], f32, tag="o")
            nc.vector.tensor_add(tot[:], s32[:], psum[:])
            nc.vector.tensor_mul(sq[:], tot[:, :, 0:K], tot[:, :, 0:K])
            nc.vector.tensor_reduce(
                red[:], sq[:], mybir.AxisListType.X, mybir.AluOpType.add
            )
            nc.vector.scalar_tensor_tensor(
                out=o[:], in0=red[:], scalar=0.5, in1=tot[:, :, K:K + 1],
                op0=mybir.AluOpType.mult, op1=mybir.AluOpType.add,
            )
            nc.sync.dma_start(y_d[:, :], o[:, :, 0])
    nc.compile()
    return nc


def _build_table(w, V):
    """[TRP, 256] bf16 pair units: [A 65 | pad | D 65 | pad]."""
    Vt = np.ascontiguousarray(V.T)  # [F, K] f32
    c = (w[:, 0] - 0.5 * np.einsum("fk,fk->f", Vt, Vt)).astype(np.float32)
    nrows = NUM_SPARSE * FEAT
    va = np.empty((nrows, ROW), dtype=np.float32)
    va[:, 0:K] = Vt[NUM_DENSE:NUM_DENSE + nrows]
    va[:, K] = c[NUM_DENSE:NUM_DENSE + nrows]
    va = va.reshape(TRP, 2, ROW)
    tab = np.zeros((TRP, 2 * UNIT), dtype=ml_dtypes.bfloat16)
    tab[:, 0:ROW] = va[:, 0].astype(ml_dtypes.bfloat16)
    tab[:, UNIT:UNIT + ROW] = (va[:, 1] - va[:, 0]).astype(ml_dtypes.bfloat16)
    return np.ascontiguousarray(tab)


def _prepare_dense(dense, w0, w, V):
    Vt_d = np.ascontiguousarray(V[:, :NUM_DENSE].T)  # [13, K]
    lhs = np.concatenate(
        [dense.T, dense.T ** 2, np.ones((1, BATCH), np.float32)], axis=0
    ).astype(np.float32)
    rhs = np.zeros((CDIM, ROW), dtype=np.float32)
    rhs[0:NUM_DENSE, 0:K] = Vt_d
    rhs[0:NUM_DENSE, K] = w[:NUM_DENSE, 0]
    rhs[NUM_DENSE:2 * NUM_DENSE, K] = -0.5 * (Vt_d ** 2).sum(axis=1)
    rhs[2 * NUM_DENSE, K] = np.asarray(w0).reshape(-1)[0]
    return lhs, rhs


def _prepare_idx(sparse):
    i = sparse.astype(np.int32)            # [BATCH, 26]
    pair = (i >> 1).astype(np.int16)
    par = (i & 1)
    gi_all, m_all = [], []
    for cidx in range(N_CORES):
        sl = slice(cidx * BPC, (cidx + 1) * BPC)
        cols = [
            np.tile(pair[sl, s].reshape(IFREE, 16).T, (8, 1))
            for s in range(NUM_SPARSE)
        ]
        gi_all.append(np.ascontiguousarray(np.concatenate(cols, axis=1)))
        # mask [128, 26, 4]: slot (p, s, t) = parity of batch row t*128+p
        pc = par[sl].reshape(NT, 128, NUM_SPARSE).transpose(1, 2, 0)
        m_all.append(np.ascontiguousarray(
            pc.astype(ml_dtypes.bfloat16).reshape(128, -1)))
    return gi_all, m_all


def kernel(dense_inputs, sparse_inputs, w0, w, V):
    dense = np.asarray(dense_inputs, dtype=np.float32)
    sparse = np.asarray(sparse_inputs)
    w0 = np.asarray(w0, dtype=np.float32)
    w = np.asarray(w, dtype=np.float32)
    V = np.asarray(V, dtype=np.float32)

    if not _nc_cache:
        _nc_cache.append(_build())
    nc = _nc_cache[0]
    fp = (w[:3, 0].tobytes(), V[:2, :3].tobytes(), float(w.sum()))
    if not _tab_cache or _tab_cache[0][0] != fp:
        _tab_cache[:] = [(fp, _build_table(w, V))]
    tab = _tab_cache[0][1]

    lhs, rhs = _prepare_dense(dense, w0, w, V)
    gi_all, m_all = _prepare_idx(sparse)

    in_maps = []
    for c in range(N_CORES):
        sl = slice(c * BPC, (c + 1) * BPC)
        dn = np.concatenate([lhs[:, sl], rhs], axis=1)
        in_maps.append({
            "tab": tab,
            "gi": gi_all[c],
            "m": m_all[c],
            "dn": np.ascontiguousarray(dn),
        })
    res = bass_utils.run_bass_kernel_spmd(
        nc, in_maps, core_ids=list(range(N_CORES)), trace=TRACE
    )
    LAST["res"] = res
    # y[p, t] on core c -> out[c*512 + t*128 + p]
    out = np.concatenate(
        [res.results[c]["y"].T.reshape(BPC, 1) for c in range(N_CORES)], axis=0
    )
    return out.astype(np.float32)
